# revision 1
# baseline (speedup 1.0000x reference)
"""Multi-head attention (B=4, T=2048, C=1024, H=16, D=64) on 8 TRN2 NeuronCores.

Sharding: data-parallel over the 4 batches x tensor-parallel over 2 head
groups (8 heads each).  Core c handles batch (c % 4), head group (c // 4).

Per-core kernel (all matmuls in fp16 inputs, fp32 PSUM accumulation):
  qT = (Wq_g x_b^T + bq_g)        [512, 2048]  (c_out on partitions)
  kT = (Wk_g x_b^T + bk_g)        [512, 2048]
  v  = (x_b Wv_g^T)               [2048, 512]  (t on partitions; bv folded on host)
  attention runs per (head-pair, 512-wide q-chunk) pass:
    S^T[tk, tq] for both heads via row-group-paired K=64 matmuls (the two
      heads sit on partition halves 0:64 / 64:128, so their matmuls occupy
      disjoint PE row groups and run concurrently)
    P = exp(S^T / 8)   one scalar-engine exp per [128, 1024] PSUM tile -> fp16
    A^T[d, tq] = sum_tk [V_h | 1] P   (M=65 matmuls: row 64 = softmax sums)
    A_h = A^T[0:64] * (1/sums)  (DVE recip-approx + gpsimd partition bcast)
  oT_partial = Wo_g^T-contraction over the 8 heads  [1024, 2048] fp32 -> HBM
  Emission interleaves QKV projections with the first attention passes so the
  scalar engine (the exp bottleneck) starts early.

Host: out[b] = (oT(b, g0) + oT(b, g1)).T + bo + Wo @ bv
(the V-bias contributes exactly Wo @ bv per row because softmax rows sum to 1).
"""

import sys

if "/opt/trn_rl_repo" not in sys.path:
    sys.path.insert(0, "/opt/trn_rl_repo")

import numpy as np
import ml_dtypes

from concourse.bacc import Bacc
import concourse.mybir as mybir
import concourse.tile as tile
from concourse.bass_utils import run_bass_kernel_spmd

F32 = mybir.dt.float32
F32R = mybir.dt.float32r
BF16 = mybir.dt.float16  # compute dtype (fp16: same PE speed as bf16, more mantissa)
EXPF = mybir.ActivationFunctionType.Exp

B, T, C = 4, 2048, 1024
H, D = 16, 64
HPC = 8          # heads per core
CS = HPC * D     # c_out slice per core = 512
NKT = T // 128   # 16 k-tiles over t_k
NQC = T // 512   # 4 q-chunks of 512
P_BUFS = 29


def build_nc():
    nc = Bacc(trn_type="TRN2")
    xT_d = nc.dram_tensor("xT", [C, T], BF16, kind="ExternalInput")
    wq_d = nc.dram_tensor("wqT", [C, CS], BF16, kind="ExternalInput")
    wk_d = nc.dram_tensor("wkT", [C, CS], BF16, kind="ExternalInput")
    wv_d = nc.dram_tensor("wvT", [C, CS], BF16, kind="ExternalInput")
    wo_d = nc.dram_tensor("woT", [CS, C], BF16, kind="ExternalInput")
    bq_d = nc.dram_tensor("bq", [CS, 1], F32, kind="ExternalInput")
    bk_d = nc.dram_tensor("bk", [CS, 1], F32, kind="ExternalInput")
    oT_d = nc.dram_tensor("oT", [C, T], F32, kind="ExternalOutput")

    with tile.TileContext(nc) as tc:
        with (
            tc.tile_pool(name="consts", bufs=1) as consts,
            tc.tile_pool(name="qkv", bufs=1) as qkv,
            tc.tile_pool(name="ptiles", bufs=P_BUFS) as ppool,
            tc.tile_pool(name="small", bufs=2) as small,
            tc.tile_pool(name="ostage", bufs=4) as ostage,
            tc.tile_pool(name="ps", bufs=2, space="PSUM") as ps,
        ):
            # ---- persistent tiles; x^T and Wq/Wk first so QKV starts early.
            # x^T is split per c_in tile so matmuls start before the whole
            # load lands.
            xts = []
            wq_cis = []
            for ci in range(8):
                wqc = consts.tile([128, CS], BF16, tag=f"wq{ci}", name=f"wq{ci}")
                nc.sync.dma_start(out=wqc, in_=wq_d[ci * 128:(ci + 1) * 128, :])
                wq_cis.append(wqc)
                xt = consts.tile([128, T], BF16, tag=f"xt{ci}", name=f"xt{ci}")
                nc.sync.dma_start(out=xt, in_=xT_d[ci * 128:(ci + 1) * 128, :])
                xts.append(xt)
            wk_cis = []
            for ci in range(8):
                wkc = consts.tile([128, CS], BF16, tag=f"wk{ci}", name=f"wk{ci}")
                nc.sync.dma_start(out=wkc, in_=wk_d[ci * 128:(ci + 1) * 128, :])
                wk_cis.append(wkc)
            wv_sb = consts.tile([128, 8, CS], BF16)
            nc.sync.dma_start(out=wv_sb, in_=wv_d[:, :].rearrange("(c p) n -> p c n", p=128))
            wo_sb = consts.tile([128, 4, C], BF16)
            nc.sync.dma_start(out=wo_sb, in_=wo_d[:, :].rearrange("(c p) n -> p c n", p=128))
            bq_sb = consts.tile([128, 4], F32)
            nc.sync.dma_start(out=bq_sb, in_=bq_d[:, :].rearrange("(c p) n -> p (c n)", p=128))
            bk_sb = consts.tile([128, 4], F32)
            nc.sync.dma_start(out=bk_sb, in_=bk_d[:, :].rearrange("(c p) n -> p (c n)", p=128))
            ones1 = consts.tile([128, 1], BF16)
            nc.vector.memset(ones1, 1.0)

            qT_sb = qkv.tile([128, 4, T], BF16)
            kT_sb = qkv.tile([128, 4, T], BF16)
            # V with a ones column appended, one tile per t_k tile so AV can
            # start as soon as the first v-projection rows land:
            # vh_tts[tt] = [tk partition, head, 64+1]
            vh_tts = []
            for tt in range(NKT):
                vht = qkv.tile([128, HPC, D + 1], BF16, tag=f"vh{tt}", name=f"vh{tt}")
                nc.vector.memset(vht[:, :, D:D + 1], 1.0)
                vh_tts.append(vht)
            # attention output, one tile per q-chunk (avoids false WAR between
            # the second-half passes and the first outproj)
            a_qcs = [qkv.tile([128, 4, 512], BF16, tag=f"a{qc}", name=f"a{qc}")
                     for qc in range(NQC)]

            # ---- QKV projection emitters ----
            def qk_proj(mt):
                # q and k output c_out tile mt; LDW shared across the 4 t-chunks
                for (w_cis, b_sb, dst) in ((wq_cis, bq_sb, qT_sb), (wk_cis, bk_sb, kT_sb)):
                    pmms = [ps.tile([128, 512], F32, tag="acc", bufs=4, name="pmm")
                            for _ in range(4)]
                    for ci in range(8):
                        for t in range(4):
                            nc.tensor.matmul(
                                pmms[t],
                                w_cis[ci][:, mt * 128:(mt + 1) * 128],
                                xts[ci][:, t * 512:(t + 1) * 512],
                                start=(ci == 0), stop=(ci == 7),
                            )
                    for t in range(4):
                        nc.vector.tensor_scalar_add(
                            dst[:, mt, t * 512:(t + 1) * 512], pmms[t], b_sb[:, mt:mt + 1]
                        )

            def v_proj():
                for tt in range(NKT):
                    pmm = ps.tile([128, 512], F32, tag="acc", bufs=4, name="pmm")
                    for ci in range(8):
                        nc.tensor.matmul(
                            pmm,
                            xts[ci][:, tt * 128:(tt + 1) * 128],
                            wv_sb[:, ci, :],
                            start=(ci == 0), stop=(ci == 7),
                        )
                    for h in range(HPC):
                        nc.vector.tensor_copy(
                            vh_tts[tt][:, h, 0:D], pmm[:, h * D:(h + 1) * D]
                        )

            # ---- attention pass = (head pair hp, q-chunk qc) ----
            # Heads A=2hp (partitions 0:64) and B=2hp+1 (64:128).  Per k-tile,
            # one [128, 1024] score PSUM tile holds A|B, produced by two
            # row-group-paired K=64 matmuls that run concurrently on the PE;
            # one exp covers both heads.  AV folds the softmax sums in as
            # matmul row 64 (ones column of vh).
            def attn_scores(hp, qc):
                ptiles = []
                q0 = qc * 512
                for kt in range(NKT):
                    pt = ppool.tile([128, 1024], BF16, tag="P", bufs=P_BUFS, name="pt")
                    ptiles.append(pt)
                    sc = ps.tile([128, 1024], F32, tag="sc", bufs=2, name="sc")
                    for hb in range(2):  # A then B, adjacent for row-group pairing
                        nc.tensor.matmul(
                            sc[:, hb * 512:(hb + 1) * 512],
                            kT_sb[hb * 64:hb * 64 + 64, hp, kt * 128:(kt + 1) * 128],
                            qT_sb[hb * 64:hb * 64 + 64, hp, q0:q0 + 512],
                            start=True, stop=True,
                        )
                    nc.scalar.activation(pt, sc, EXPF, scale=0.125)
                return ptiles

            def attn_av(hp, qc, ptiles):
                avs = [ps.tile([65, 512], F32, tag="acc", bufs=4, name="av")
                       for _ in range(2)]
                for kt in range(NKT):
                    st, sp = kt == 0, kt == NKT - 1
                    for hb in range(2):
                        nc.tensor.matmul(
                            avs[hb],
                            vh_tts[kt][:, 2 * hp + hb, :],
                            ptiles[kt][:, hb * 512:(hb + 1) * 512],
                            start=st, stop=sp,
                        )
                for hb in range(2):
                    av = avs[hb]
                    ssum = small.tile([1, 512], F32, tag="ssum", bufs=1, name="ssum")
                    nc.vector.tensor_copy(ssum, av[64:65, :])
                    rec = small.tile([1, 512], F32, tag="rec", bufs=1, name="rec")
                    nc.vector.reciprocal_approx_fast(out=rec, in_=ssum)
                    rbs = small.tile([64, 512], F32, tag="rbs", bufs=1, name="rbs")
                    nc.gpsimd.partition_broadcast(rbs, rec)
                    nc.vector.tensor_mul(
                        a_qcs[qc][hb * 64:hb * 64 + 64, hp, :],
                        av[0:64, :], rbs,
                    )

            def outproj_qc(qc):
                # oT[c_out, t] for one 512-wide q-chunk
                for mt in range(8):
                    po = ps.tile([128, 512], F32, tag="acc", bufs=4, name="po")
                    for ci in range(4):
                        nc.tensor.matmul(
                            po,
                            wo_sb[:, ci, mt * 128:(mt + 1) * 128],
                            a_qcs[qc][:, ci, :],
                            start=(ci == 0), stop=(ci == 3),
                        )
                    ot = ostage.tile([128, 512], F32, tag="ot", bufs=7, name="ot")
                    nc.vector.tensor_copy(ot, po)
                    nc.sync.dma_start(
                        out=oT_d[mt * 128:(mt + 1) * 128, qc * 512:(qc + 1) * 512],
                        in_=ot,
                    )

            # ---- emission schedule: interleave QKV with early attention so
            # the scalar engine (exp) starts as soon as possible ----
            qk_proj(0)
            pt00 = attn_scores(0, 0)
            qk_proj(1)
            pt10 = attn_scores(1, 0)
            v_proj()
            pt01 = attn_scores(0, 1)
            attn_av(0, 0, pt00)
            pt11 = attn_scores(1, 1)
            attn_av(1, 0, pt10)
            qk_proj(2)
            pt20 = attn_scores(2, 0)
            attn_av(0, 1, pt01)
            pt21 = attn_scores(2, 1)
            attn_av(1, 1, pt11)
            qk_proj(3)
            pt30 = attn_scores(3, 0)
            attn_av(2, 0, pt20)
            pt31 = attn_scores(3, 1)
            attn_av(2, 1, pt21)
            tail = [(hp, qc) for qc in (2, 3) for hp in range(4)]
            prev = attn_scores(*tail[0])
            attn_av(3, 0, pt30)
            outproj_qc(0)
            nxt = attn_scores(*tail[1])
            attn_av(3, 1, pt31)
            outproj_qc(1)
            attn_av(*tail[0], prev)
            prev = nxt
            for i in range(2, len(tail)):
                nxt = attn_scores(*tail[i])
                attn_av(*tail[i - 1], prev)
                if tail[i - 1] == (3, 2):
                    outproj_qc(2)
                prev = nxt
            attn_av(*tail[-1], prev)
            outproj_qc(3)
    nc.finalize()
    return nc


_NC = None


def _get_nc():
    global _NC
    if _NC is None:
        _NC = build_nc()
    return _NC


def _shard_inputs(x, Wq, bq, Wk, bk, Wv, bv, Wo, bo):
    bf = np.float16
    x = np.asarray(x, np.float32)
    in_maps = []
    wqT = np.ascontiguousarray(np.asarray(Wq, np.float32).T).astype(bf)  # [C, C] = [c_in, c_out]
    wkT = np.ascontiguousarray(np.asarray(Wk, np.float32).T).astype(bf)
    wvT = np.ascontiguousarray(np.asarray(Wv, np.float32).T).astype(bf)
    woT = np.ascontiguousarray(np.asarray(Wo, np.float32).T).astype(bf)  # [c_in, c_out]
    xT = [np.ascontiguousarray(x[b].T).astype(bf) for b in range(B)]
    for c in range(8):
        b, g = c % B, c // B
        sl = slice(g * CS, (g + 1) * CS)
        in_maps.append({
            "xT": xT[b],
            "wqT": np.ascontiguousarray(wqT[:, sl]),
            "wkT": np.ascontiguousarray(wkT[:, sl]),
            "wvT": np.ascontiguousarray(wvT[:, sl]),
            "woT": np.ascontiguousarray(woT[sl, :]),
            "bq": np.ascontiguousarray(np.asarray(bq, np.float32)[sl]).reshape(CS, 1),
            "bk": np.ascontiguousarray(np.asarray(bk, np.float32)[sl]).reshape(CS, 1),
        })
    return in_maps


def run_sharded(inputs, **kwargs):
    """Run the SPMD kernel; returns the BassKernelResults."""
    nc = _get_nc()
    in_maps = _shard_inputs(**inputs)
    return run_bass_kernel_spmd(nc, in_maps, core_ids=list(range(8)), **kwargs)


def assemble(results, Wv_bias, Wo, bo):
    bo_eff = (np.asarray(bo, np.float32)
              + np.asarray(Wo, np.float32) @ np.asarray(Wv_bias, np.float32))
    out = np.empty((B, T, C), np.float32)
    for b in range(B):
        acc = results[b]["oT"].astype(np.float32) + results[b + B]["oT"].astype(np.float32)
        out[b] = acc.T + bo_eff[None, :]
    return out


def kernel(**inputs):
    res = run_sharded(inputs)
    return assemble(res.results, inputs["bv"], inputs["Wo"], inputs["bo"])



# revision 37
# speedup vs baseline: 1.2456x; 1.2456x over previous
"""Multi-head attention (B=4, T=2048, C=1024, H=16, D=64) on 8 TRN2 NeuronCores.

Sharding: data-parallel over the 4 batches x tensor-parallel over 2 head
groups (8 heads each).  Core c handles batch (c % 4), head group (c // 4).

Per-core kernel (fp8-e4m3 DoubleRow matmuls + multi-engine softmax exp):
  QKV projections: 3-term fp8 residual product  x_hi*W_A + x_hi*W_B + x_lo*W_A
    (W_A = e4m3(64 W), W_B = e4m3(64 W - W_A); DoubleRow, 2 K-slabs/instr)
    -> PSUM, then *2^-6 (+bq for q) quantize: q,k -> e4m3, v -> fp16.
  Scores: fp8 DoubleRow with the head dim split in two 32-slabs
    (lhsT = kT [32,2,128], rhs = qT [32,2,256], explicit tile_position)
    -> S^T [tk, tq] PSUM tiles.
  exp(S/8) -> P fp16, split across engines per tile (EXP_SCHED):
    'A' Act exact | 'L' Pool affine + DVE custom-correct | 'D' DVE both |
    'S' Pool pure Schraudolph (int16 bitcast fp16; ~2% rms, small share).
  AV (flipped): out[tq, d+1] over tk: lhsT = P [tk, tq-128], rhs = [V|1] fp16
    -> av PSUM [128, 65]; col 64 = softmax denominators.
  Normalize: DVE recip-approx + per-partition tensor_scalar -> A fp16.
  PE-transpose A -> A^T [hd, tq] (identity matmul), outproj fp16 -> oT fp16.

Host: out[b] = (oT(b,g0) + oT(b,g1)).T + bo + Wo @ bv   (bv folds to Wo@bv
because softmax rows sum to 1; bk drops: its score term is softmax-constant).
"""

import sys
import math

if "/opt/trn_rl_repo" not in sys.path:
    sys.path.insert(0, "/opt/trn_rl_repo")

import numpy as np
import ml_dtypes

import concourse.dve_ops as dve_ops
from concourse.dve_ops import (
    DveOp,
    OPS,
    CUSTOM_DVE_SPECS,
    _SUB_OPCODE_FOR_NAME,
    _CUSTOM_DVE_ROW_BASE,
)
from concourse.dve_spec import Spec, Src0, Src1, C0, C1, C2
from concourse.bacc import Bacc
import concourse.mybir as mybir
import concourse.tile as tile
from concourse.bass_utils import run_bass_kernel_spmd
from concourse import masks

F32 = mybir.dt.float32
F16 = mybir.dt.float16
FP8 = mybir.dt.float8e4
I16 = mybir.dt.int16
E4M3 = ml_dtypes.float8_e4m3
EXPF = mybir.ActivationFunctionType.Exp
IDENT = mybir.ActivationFunctionType.Identity
DR = mybir.MatmulPerfMode.DoubleRow
MULT = mybir.AluOpType.mult
ADD = mybir.AluOpType.add

B, T, C = 4, 2048, 1024
H, D = 16, 64
HPC = 8            # heads per core
CS = HPC * D       # c_out slice per core = 512
NKT = T // 128     # 16 k-tiles
NQC = T // 256     # 8 q-chunks of 256
WSC = 64.0         # weight prescale for e4m3 (W sigma 0.02 -> 1.28)

# --- custom DVE op: correct the fp16 Schraudolph exp ------------------------
K_C = float(2**33 - 512)
A_C = 1.2543697555e-05
B_C = 2.2290210725e-07
EXP1_SCALE = 0.125 * 1024.0 / math.log(2.0)
EXP1_BIAS = 15360.0
PURE_BIAS = EXP1_BIAS - 0.5 * (1.0 - (math.log(math.log(2.0)) + 1.0) / math.log(2.0)) * 1024.0

_m = Src0 - ((Src0 + C0) - C0)


def _exp_corr_ref(in0, in1, s0, s1, imm2):
    u = in0.astype(np.float32)
    t1 = (u + np.float32(s0)).astype(np.float32)
    t2 = (t1 - np.float32(s0)).astype(np.float32)
    d = (u - t2).astype(np.float32)
    y = in1.astype(np.float32)
    return (y + y * (d * (np.float32(s1) + np.float32(imm2) * d))).astype(np.float32)


EXP_CORR = DveOp(
    "EXP_CORR_ANT",
    Spec(body=Src1 + Src1 * (_m * (C1 + C2 * _m)), reference=_exp_corr_ref),
    subdim=False,
    uops_sha={"v3": "deea41a0c12878a1", "v4": "7356cbf66bc5e904"},
)
if EXP_CORR.name not in _SUB_OPCODE_FOR_NAME:
    OPS.append(EXP_CORR)
    _SUB_OPCODE_FOR_NAME[EXP_CORR.name] = _CUSTOM_DVE_ROW_BASE + len(OPS) - 1
    CUSTOM_DVE_SPECS[EXP_CORR.name] = EXP_CORR.spec

# exp engine schedule, consumed round-robin per [128,512] score tile:
#  'A' Act exact | 'L' Pool op1 + DVE op2 | 'D' DVE op1+op2 |
#  'S' Pool pure Schraudolph | 'T' DVE pure Schraudolph
EXP_SCHED = "ATATATATATATATAA" "TATATATATATATATA"
# engines for psum->sbuf quantize / copies: round robin over these
QUANT_Q_ENGS = "svsv"       # per q proj tile: v=DVE s=Act
QUANT_K_ENGS = "vsvs"
PO_ENGS = "ssssssvv"        # po copies: s=Act v=DVE
AT_ENGS = "ss"              # at copy per qc
NORM_ENGS = "vv"            # norm tensor_scalar per (h,tqc)


def build_nc():
    nc = Bacc(trn_type="TRN2")
    xh_d = nc.dram_tensor("xh", [C, T], FP8, kind="ExternalInput")
    xl_d = nc.dram_tensor("xl", [C, T], FP8, kind="ExternalInput")
    w_ds = {}
    for nm in ("qA", "qB", "kA", "kB", "vA", "vB"):
        w_ds[nm] = nc.dram_tensor(f"w{nm}", [128, 4 * 2 * CS], FP8, kind="ExternalInput")
    wo_d = nc.dram_tensor("woT", [128, 4 * C], F16, kind="ExternalInput")
    bq_d = nc.dram_tensor("bq", [128, 4], F32, kind="ExternalInput")
    oT_d = nc.dram_tensor("oT", [C, T], F16, kind="ExternalOutput")

    with tile.TileContext(nc) as tc:
        with (
            tc.tile_pool(name="consts", bufs=1) as consts,
            tc.tile_pool(name="qkv", bufs=1) as qkv,
            tc.tile_pool(name="ppool", bufs=40) as ppool,
            tc.tile_pool(name="asb", bufs=1) as asb,
            tc.tile_pool(name="small", bufs=2) as small,
            tc.tile_pool(name="ps", bufs=1, space="PSUM") as ps,
        ):
            ENG = {"v": nc.vector, "g": nc.gpsimd}

            # ---------------- persistent inputs ----------------
            xh_sb, xl_sb, w_sb = [], [], {}

            def dma_w(nm, eng):
                w = consts.tile([128, 4, 2, CS], FP8, tag=f"w{nm}", name=f"w{nm}")
                eng.dma_start(out=w, in_=w_ds[nm].rearrange("p (a b n) -> p a b n", a=4, b=2))
                w_sb[nm] = w

            def dma_x(pr):
                # xh on the SP hwdge queue, xl on the Act queue (parallel)
                xh = consts.tile([128, 2, T], FP8, tag=f"xh{pr}", name=f"xh{pr}")
                nc.sync.dma_start(out=xh, in_=xh_d[pr * 256:(pr + 1) * 256, :]
                                  .rearrange("(b p) n -> p b n", p=128))
                xh_sb.append(xh)
                xl = consts.tile([128, 2, T], FP8, tag=f"xl{pr}", name=f"xl{pr}")
                nc.scalar.dma_start(out=xl, in_=xl_d[pr * 256:(pr + 1) * 256, :]
                                    .rearrange("(b p) n -> p b n", p=128))
                xl_sb.append(xl)

            dma_w("qA", nc.sync); dma_w("qB", nc.scalar)
            dma_x(0); dma_x(1)
            dma_w("kA", nc.sync); dma_w("kB", nc.scalar)
            dma_x(2); dma_x(3)
            dma_w("vA", nc.sync); dma_w("vB", nc.scalar)
            wo_sb = consts.tile([128, 4, C], F16)
            nc.sync.dma_start(out=wo_sb, in_=wo_d.rearrange("p (a n) -> p a n", a=4))
            bq_sb = consts.tile([128, 4], F32)
            nc.sync.dma_start(out=bq_sb, in_=bq_d[:, :])
            ident = consts.tile([128, 128], F16)
            masks.make_identity(nc, ident[:, :])

            qT = [qkv.tile([128, 2, T], FP8, tag=f"qT{i}", name=f"qT{i}") for i in range(2)]
            kT = [qkv.tile([128, 2, T], FP8, tag=f"kT{i}", name=f"kT{i}") for i in range(2)]
            vh = [qkv.tile([128, 2, HPC, D + 1], F16, tag=f"vh{i}", name=f"vh{i}")
                  for i in range(NKT // 2)]
            for v in vh:
                nc.vector.memset(v[:, :, :, D:D + 1], 1.0)
            a_sb = [asb.tile([128, HPC, D], F16, tag=f"a{i}", name=f"a{i}")
                    for i in range(4)]
            at_sb = [asb.tile([128, 4, 256], F16, tag=f"at{i}", name=f"at{i}")
                     for i in range(2)]
            ost_sb = [asb.tile([128, 8, 256], F16, tag=f"ost{i}", name=f"ost{i}")
                      for i in range(2)]

            # ---------------- projections ----------------
            def proj(dst_kind, mt):
                # pr-outer so matmuls start as soon as x chunk pr lands
                wa, wb = w_sb[dst_kind + "A"], w_sb[dst_kind + "B"]
                engs = QUANT_Q_ENGS if dst_kind == "q" else QUANT_K_ENGS
                for t in range(4):
                    pmm = ps.tile([128, 512], F32, tag="sc", bufs=4, name="pmm")
                    n = 0
                    for (xx, ww) in ((xh_sb, wa), (xh_sb, wb), (xl_sb, wa)):
                        for pr in range(4):
                            nc.tensor.matmul(
                                pmm,
                                ww[:, pr, :, mt * 128:(mt + 1) * 128],
                                xx[pr][:, :, t * 512:(t + 1) * 512],
                                start=(n == 0), stop=(n == 11),
                                perf_mode=DR,
                            )
                            n += 1
                    sl = slice(t * 512, (t + 1) * 512)
                    dst = (qT if dst_kind == "q" else kT)[mt // 2][:, mt % 2, sl]
                    if engs[t] == "s":
                        if dst_kind == "q":
                            nc.scalar.activation(dst, pmm, IDENT,
                                                 bias=bq_sb[:, mt:mt + 1], scale=1.0 / WSC)
                        else:
                            nc.scalar.mul(dst, pmm, 1.0 / WSC)
                    elif dst_kind == "q":
                        nc.vector.tensor_scalar(dst, pmm, 1.0 / WSC,
                                                bq_sb[:, mt:mt + 1], MULT, ADD)
                    else:
                        nc.vector.tensor_scalar(dst, pmm, 1.0 / WSC, None, MULT)

            def v_proj(tt):
                pmm = ps.tile([128, 512], F32, tag="sc", bufs=4, name="pmv")
                n = 0
                for (xx, wk) in ((xh_sb, "vA"), (xh_sb, "vB"), (xl_sb, "vA")):
                    for pr in range(4):
                        nc.tensor.matmul(
                            pmm,
                            xx[pr][:, :, tt * 128:(tt + 1) * 128],
                            w_sb[wk][:, pr, :, :],
                            start=(n == 0), stop=(n == 11),
                            perf_mode=DR,
                        )
                        n += 1
                if tt % 2 == 0:
                    nc.scalar.mul(vh[tt // 2][:, tt % 2, :, 0:D],
                                  pmm.rearrange("p (a b) -> p a b", a=HPC), 1.0 / WSC)
                else:
                    nc.vector.tensor_scalar(
                        vh[tt // 2][:, tt % 2, :, 0:D],
                        pmm.rearrange("p (a b) -> p a b", a=HPC), 1.0 / WSC, None, MULT)

            # ---------------- attention ----------------
            exp_ctr = [0]

            def scores_exp(h, qc):
                """8 score tiles [128,512] (one kt-pair each) + exp -> 8 P
                tiles [128, 2, 256] per (head, qc256)."""
                ti, rb = (0, 32 * h) if h < 4 else (1, 32 * (h - 4))
                qsl = qT[ti][rb:rb + 32, :, qc * 256:(qc + 1) * 256]
                ptiles = []
                for kp in range(8):
                    sc = ps.tile([128, 512], F32, tag="sc", bufs=4, name="sc")
                    for k2 in range(2):
                        kt = kp * 2 + k2
                        nc.tensor.matmul(
                            sc[:, k2 * 256:(k2 + 1) * 256],
                            kT[ti][rb:rb + 32, :, kt * 128:(kt + 1) * 128],
                            qsl, start=True, stop=True, perf_mode=DR,
                            tile_position=(rb, 0),
                        )
                    pt = ppool.tile([128, 2, 256], F16, tag="P", bufs=80, name="pt")
                    ptiles.append(pt)
                    eng = EXP_SCHED[exp_ctr[0] % len(EXP_SCHED)]
                    exp_ctr[0] += 1
                    pf = pt.rearrange("p a b -> p (a b)")
                    if eng == "A":
                        nc.scalar.activation(pf, sc, EXPF, scale=0.125)
                    else:
                        u = pf.bitcast(I16)
                        if eng == "S":
                            nc.gpsimd.tensor_scalar(u, sc, EXP1_SCALE, PURE_BIAS, MULT, ADD)
                        elif eng == "T":
                            nc.vector.tensor_scalar(u, sc, EXP1_SCALE, PURE_BIAS, MULT, ADD)
                        else:
                            e = nc.gpsimd if eng == "L" else nc.vector
                            e.tensor_scalar(u, sc, EXP1_SCALE, EXP1_BIAS, MULT, ADD)
                            nc.vector._custom_dve(EXP_CORR, out=pf, in0=u, in1=pf,
                                                  s0=K_C, s1=A_C, imm2=B_C)
                return ptiles

            def av_chunk(pts, h, hq, qc, tqc, hi):
                av = ps.tile([128, D + 1], F32, tag="av", bufs=2, name="av")
                for kt in range(NKT):
                    nc.tensor.matmul(
                        av,
                        pts[kt // 2][:, kt % 2, tqc * 128:(tqc + 1) * 128],
                        vh[kt // 2][:, kt % 2, h, :],
                        start=(kt == 0), stop=(kt == NKT - 1),
                    )
                # normalize: approx-reciprocal of sums then per-partition mult
                rec = small.tile([128, 1], F32, tag="rec", bufs=6, name="rec")
                nc.vector.reciprocal_approx_fast(out=rec, in_=av[:, D:D + 1])
                nc.vector.tensor_scalar(
                    a_sb[(qc % 2) * 2 + tqc][:, h, :],
                    av[:, 0:D], rec, None, MULT)

            def transpose_at(qc):
                at = ps.tile([128, 4, 256], F16, tag="opo", bufs=2, name="at")
                for hp in range(4):
                    for hi in range(2):
                        h = hp * 2 + hi
                        for tqc in range(2):
                            nc.tensor.transpose(
                                at[hi * 64:hi * 64 + 64, hp, tqc * 128:(tqc + 1) * 128],
                                a_sb[(qc % 2) * 2 + tqc][:, h, :],
                                ident[:, :])
                if AT_ENGS[qc % 2] == "s":
                    nc.scalar.copy(at_sb[qc % 2], at)
                else:
                    nc.vector.tensor_copy(at_sb[qc % 2], at)

            def po_chunk(qc, mt):
                po = ps.tile([128, 256], F32, tag="opo", bufs=2, name="po")
                for hp in range(4):
                    nc.tensor.matmul(
                        po,
                        wo_sb[:, hp, mt * 128:(mt + 1) * 128],
                        at_sb[qc % 2][:, hp, :],
                        start=(hp == 0), stop=(hp == 3),
                    )
                e = PO_ENGS[(qc * 8 + mt) % len(PO_ENGS)]
                if e == "s":
                    nc.scalar.copy(ost_sb[qc % 2][:, mt, :], po)
                else:
                    ENG[e].tensor_copy(ost_sb[qc % 2][:, mt, :], po)
                if mt == 7:
                    nc.sync.dma_start(
                        out=oT_d[:, qc * 256:(qc + 1) * 256]
                        .rearrange("(a p) n -> p a n", p=128),
                        in_=ost_sb[qc % 2])

            # ---------------- emission schedule ----------------
            proj("q", 0); proj("q", 1)
            proj("k", 0); proj("k", 1)
            u0 = [scores_exp(hi, 0) for hi in range(4)]          # qc0, hq0
            proj("q", 2); proj("q", 3)
            proj("k", 2); proj("k", 3)
            u1 = [scores_exp(4 + hi, 0) for hi in range(4)]      # qc0, hq1
            for tt in range(NKT):
                v_proj(tt)

            # staggered pipeline, finely woven: consumer chunks (AV/norm,
            # transposes, outproj) of earlier units are interleaved between
            # the per-head score bursts of the current unit, so the in-order
            # PE always has work while the exp engines drain score PSUM.
            def consumers_for(done, out_qc):
                quad_p, hq, qc = done
                thunks = []
                for tqc in range(2):
                    for hi in range(4):
                        thunks.append(lambda p=quad_p[hi], h=hq * 4 + hi, hq_=hq,
                                      qc_=qc, t_=tqc, hi_=hi:
                                      av_chunk(p, h, hq_, qc_, t_, hi_))
                if hq == 1:
                    thunks.append(lambda qc_=qc: transpose_at(qc_))
                if out_qc is not None:
                    for mt in range(8):
                        thunks.append(lambda q_=out_qc, m_=mt: po_chunk(q_, m_))
                return thunks

            units = [(qc, hq) for qc in range(NQC) for hq in range(2)]
            pend = [(u0, 0, 0), (u1, 1, 0)]
            pend_out = []
            for (qc, hq) in units[2:]:
                done = pend.pop(0)
                out_qc = pend_out.pop(0) if pend_out else None
                cons = consumers_for(done, out_qc)
                if done[1] == 1:
                    pend_out.append(done[2])
                quad_p = []
                n = len(cons)
                for hi in range(4):
                    take = (n * (hi + 1)) // 4 - (n * hi) // 4
                    for _ in range(take):
                        cons.pop(0)()
                    quad_p.append(scores_exp(hq * 4 + hi, qc))
                pend.append((quad_p, hq, qc))
            for done in pend:
                out_qc = pend_out.pop(0) if pend_out else None
                for th in consumers_for(done, out_qc):
                    th()
                if done[1] == 1:
                    pend_out.append(done[2])
            for qc in pend_out:
                for mt in range(8):
                    po_chunk(qc, mt)
    nc.finalize()
    return nc


_NC = None


def _get_nc():
    global _NC
    if _NC is None:
        _NC = build_nc()
    return _NC


def _dsplit_perm():
    perm = np.empty(CS, np.int64)
    i = 0
    for mt in range(4):
        hbase, half = (0 if mt < 2 else 4), mt % 2
        for h in range(4):
            for dd in range(32):
                perm[i] = (hbase + h) * D + half * 32 + dd
                i += 1
    return perm


_PERM = _dsplit_perm()


def _w_pair_layout(w):
    return np.ascontiguousarray(
        w.reshape(4, 2, 128, CS).transpose(2, 0, 1, 3).reshape(128, 4 * 2 * CS))


def _quant_w(wT):
    wa = (WSC * wT).astype(E4M3)
    wb = (WSC * wT - wa.astype(np.float32)).astype(E4M3)
    return _w_pair_layout(wa), _w_pair_layout(wb)


def _shard_inputs(x, Wq, bq, Wk, bk, Wv, bv, Wo, bo):
    x = np.asarray(x, np.float32)
    wqT = np.ascontiguousarray(np.asarray(Wq, np.float32).T)
    wkT = np.ascontiguousarray(np.asarray(Wk, np.float32).T)
    wvT = np.ascontiguousarray(np.asarray(Wv, np.float32).T)
    woT = np.ascontiguousarray(np.asarray(Wo, np.float32).T)
    bqf = np.asarray(bq, np.float32)

    xh_b, xl_b = [], []
    for b in range(B):
        xT = np.ascontiguousarray(x[b].T)
        xh = xT.astype(E4M3)
        xl = (xT - xh.astype(np.float32)).astype(E4M3)
        xh_b.append(xh); xl_b.append(xl)

    in_maps = []
    for c in range(8):
        b, g = c % B, c // B
        sl = slice(g * CS, (g + 1) * CS)
        qa, qb = _quant_w(wqT[:, sl][:, _PERM])
        ka, kb = _quant_w(wkT[:, sl][:, _PERM])
        va, vb_ = _quant_w(wvT[:, sl])
        wos = np.ascontiguousarray(
            woT[sl, :].reshape(4, 128, C).transpose(1, 0, 2)
            .reshape(128, 4 * C)).astype(np.float16)
        bqp = np.ascontiguousarray(
            bqf[sl][_PERM].reshape(4, 128).T).astype(np.float32)
        in_maps.append({
            "xh": xh_b[b], "xl": xl_b[b],
            "wqA": qa, "wqB": qb, "wkA": ka, "wkB": kb, "wvA": va, "wvB": vb_,
            "woT": wos, "bq": bqp,
        })
    return in_maps


def run_sharded(inputs, **kwargs):
    nc = _get_nc()
    in_maps = _shard_inputs(**inputs)
    return run_bass_kernel_spmd(nc, in_maps, core_ids=list(range(8)), **kwargs)


def assemble(results, Wv_bias, Wo, bo):
    bo_eff = (np.asarray(bo, np.float32)
              + np.asarray(Wo, np.float32) @ np.asarray(Wv_bias, np.float32))
    out = np.empty((B, T, C), np.float32)
    for b in range(B):
        acc = results[b]["oT"].astype(np.float32) + results[b + B]["oT"].astype(np.float32)
        out[b] = acc.T + bo_eff[None, :]
    return out


def kernel(**inputs):
    res = run_sharded(inputs)
    return assemble(res.results, inputs["bv"], inputs["Wo"], inputs["bo"])


# revision 43
# speedup vs baseline: 1.2563x; 1.0086x over previous
"""Multi-head attention (B=4, T=2048, C=1024, H=16, D=64) on 8 TRN2 NeuronCores.

Sharding: data-parallel over the 4 batches x tensor-parallel over 2 head
groups (8 heads each).  Core c handles batch (c % 4), head group (c // 4).

Per-core kernel (fp8-e4m3 DoubleRow matmuls + multi-engine softmax exp):
  QKV projections: 3-term fp8 residual product  x_hi*W_A + x_hi*W_B + x_lo*W_A
    (W_A = e4m3(64 W), W_B = e4m3(64 W - W_A); DoubleRow, 2 K-slabs/instr)
    -> PSUM, then *2^-6 (+bq for q) quantize: q,k -> e4m3, v -> fp16.
  Scores: fp8 DoubleRow with the head dim split in two 32-slabs
    (lhsT = kT [32,2,128], rhs = qT [32,2,256], explicit tile_position)
    -> S^T [tk, tq] PSUM tiles.
  exp(S/8) -> P fp16, split across engines per tile (EXP_SCHED):
    'A' Act exact | 'L' Pool affine + DVE custom-correct | 'D' DVE both |
    'S' Pool pure Schraudolph (int16 bitcast fp16; ~2% rms, small share).
  AV (flipped): out[tq, d+1] over tk: lhsT = P [tk, tq-128], rhs = [V|1] fp16
    -> av PSUM [128, 65]; col 64 = softmax denominators.
  Normalize: DVE recip-approx + per-partition tensor_scalar -> A fp16.
  PE-transpose A -> A^T [hd, tq] (identity matmul), outproj fp16 -> oT fp16.

Host: out[b] = (oT(b,g0) + oT(b,g1)).T + bo + Wo @ bv   (bv folds to Wo@bv
because softmax rows sum to 1; bk drops: its score term is softmax-constant).
"""

import sys
import math

if "/opt/trn_rl_repo" not in sys.path:
    sys.path.insert(0, "/opt/trn_rl_repo")

import numpy as np
import ml_dtypes

import concourse.dve_ops as dve_ops
from concourse.dve_ops import (
    DveOp,
    OPS,
    CUSTOM_DVE_SPECS,
    _SUB_OPCODE_FOR_NAME,
    _CUSTOM_DVE_ROW_BASE,
)
from concourse.dve_spec import Spec, Src0, Src1, C0, C1, C2
from concourse.bacc import Bacc
import concourse.mybir as mybir
import concourse.tile as tile
from concourse.bass_utils import run_bass_kernel_spmd
from concourse import masks

F32 = mybir.dt.float32
F16 = mybir.dt.float16
FP8 = mybir.dt.float8e4
I16 = mybir.dt.int16
E4M3 = ml_dtypes.float8_e4m3
EXPF = mybir.ActivationFunctionType.Exp
IDENT = mybir.ActivationFunctionType.Identity
DR = mybir.MatmulPerfMode.DoubleRow
MULT = mybir.AluOpType.mult
ADD = mybir.AluOpType.add

B, T, C = 4, 2048, 1024
H, D = 16, 64
HPC = 8            # heads per core
CS = HPC * D       # c_out slice per core = 512
NKT = T // 128     # 16 k-tiles
NQC = T // 256     # 8 q-chunks of 256
WSC = 64.0         # weight prescale for e4m3 (W sigma 0.02 -> 1.28)

# --- custom DVE op: correct the fp16 Schraudolph exp ------------------------
K_C = float(2**33 - 512)
A_C = 1.2543697555e-05
B_C = 2.2290210725e-07
EXP1_SCALE = 0.125 * 1024.0 / math.log(2.0)
EXP1_BIAS = 15360.0
PURE_BIAS = EXP1_BIAS - 0.5 * (1.0 - (math.log(math.log(2.0)) + 1.0) / math.log(2.0)) * 1024.0

_m = Src0 - ((Src0 + C0) - C0)

# --- fused normalize: out = Src0 * approx-recip(Src1), 1-NR (6 stages) ------
NORM_C0 = -0.23620000
NORM_C1 = 2.00250000
from concourse.dve_spec import Bin, AluOp as _DveAluOp
_ny0 = Bin(_DveAluOp.BITWISE_NOT, Src1, Src1) * C0
_nrec = _ny0 * (C1 - Src1 * _ny0)


def _norm_mul_ref(in0, in1, s0, s1, imm2):
    x = np.broadcast_to(in1, in0.shape).astype(np.float32)
    nx = (~x.view(np.int32)).view(np.float32)
    y0 = (nx * np.float32(s0)).astype(np.float32)
    rec = (y0 * (np.float32(s1) - x * y0)).astype(np.float32)
    return (in0.astype(np.float32) * rec).astype(np.float32)


NORM_MUL = DveOp(
    "NORM_RECIP_MUL_ANT",
    Spec(body=Src0 * _nrec, reference=_norm_mul_ref),
    subdim=False,
    uops_sha={},
)


def _exp_corr_ref(in0, in1, s0, s1, imm2):
    u = in0.astype(np.float32)
    t1 = (u + np.float32(s0)).astype(np.float32)
    t2 = (t1 - np.float32(s0)).astype(np.float32)
    d = (u - t2).astype(np.float32)
    y = in1.astype(np.float32)
    return (y + y * (d * (np.float32(s1) + np.float32(imm2) * d))).astype(np.float32)


EXP_CORR = DveOp(
    "EXP_CORR_ANT",
    Spec(body=Src1 + Src1 * (_m * (C1 + C2 * _m)), reference=_exp_corr_ref),
    subdim=False,
    uops_sha={"v3": "deea41a0c12878a1", "v4": "7356cbf66bc5e904"},
)
if EXP_CORR.name not in _SUB_OPCODE_FOR_NAME:
    OPS.append(EXP_CORR)
    _SUB_OPCODE_FOR_NAME[EXP_CORR.name] = _CUSTOM_DVE_ROW_BASE + len(OPS) - 1
    CUSTOM_DVE_SPECS[EXP_CORR.name] = EXP_CORR.spec
if NORM_MUL.name not in _SUB_OPCODE_FOR_NAME:
    OPS.append(NORM_MUL)
    _SUB_OPCODE_FOR_NAME[NORM_MUL.name] = _CUSTOM_DVE_ROW_BASE + len(OPS) - 1
    CUSTOM_DVE_SPECS[NORM_MUL.name] = NORM_MUL.spec
    try:
        NORM_MUL.compile("v3")
    except ValueError as _e:  # pin the sha that lower() reports
        _sha = str(_e).split('"v3"]="')[1].split('"')[0]
        object.__setattr__(NORM_MUL, "uops_sha", {"v3": _sha})
        dve_ops._COMPILE_CACHE.pop((NORM_MUL.name, "v3"), None)

# exp engine schedule, consumed round-robin per [128,512] score tile:
#  'A' Act exact | 'L' Pool op1 + DVE op2 | 'D' DVE op1+op2 |
#  'S' Pool pure Schraudolph | 'T' DVE pure Schraudolph
EXP_SCHED = "ATATATATATATATAA" "TATATATATATATATA"
# engines for psum->sbuf quantize / copies: round robin over these
QUANT_Q_ENGS = "svsv"       # per q proj tile: v=DVE s=Act
QUANT_K_ENGS = "vsvs"
PO_ENGS = "ssvvssvv"        # po copies: s=Act v=DVE
AT_ENGS = "ss"              # at copy per qc
NORM_ENGS = "vv"            # norm tensor_scalar per (h,tqc)


def build_nc():
    nc = Bacc(trn_type="TRN2")
    xh_d = nc.dram_tensor("xh", [C, T], FP8, kind="ExternalInput")
    xl_d = nc.dram_tensor("xl", [C, T], FP8, kind="ExternalInput")
    w_ds = {}
    for nm in ("qA", "qB", "kA", "kB", "vA", "vB"):
        w_ds[nm] = nc.dram_tensor(f"w{nm}", [128, 4 * 2 * CS], FP8, kind="ExternalInput")
    wo_d = nc.dram_tensor("woT", [128, 4 * C], F16, kind="ExternalInput")
    bq_d = nc.dram_tensor("bq", [128, 4], F32, kind="ExternalInput")
    oT_d = nc.dram_tensor("oT", [C, T], F16, kind="ExternalOutput")

    with tile.TileContext(nc) as tc:
        with (
            tc.tile_pool(name="consts", bufs=1) as consts,
            tc.tile_pool(name="qkv", bufs=1) as qkv,
            tc.tile_pool(name="ppool", bufs=40) as ppool,
            tc.tile_pool(name="asb", bufs=1) as asb,
            tc.tile_pool(name="small", bufs=2) as small,
            tc.tile_pool(name="ps", bufs=1, space="PSUM") as ps,
        ):
            ENG = {"v": nc.vector, "g": nc.gpsimd}

            # ---------------- persistent inputs ----------------
            xh_sb, xl_sb, w_sb = [], [], {}

            def dma_w(nm, eng):
                w = consts.tile([128, 4, 2, CS], FP8, tag=f"w{nm}", name=f"w{nm}")
                eng.dma_start(out=w, in_=w_ds[nm].rearrange("p (a b n) -> p a b n", a=4, b=2))
                w_sb[nm] = w

            def dma_x(pr):
                # xh on the SP hwdge queue, xl on the Act queue (parallel)
                xh = consts.tile([128, 2, T], FP8, tag=f"xh{pr}", name=f"xh{pr}")
                nc.sync.dma_start(out=xh, in_=xh_d[pr * 256:(pr + 1) * 256, :]
                                  .rearrange("(b p) n -> p b n", p=128))
                xh_sb.append(xh)
                xl = consts.tile([128, 2, T], FP8, tag=f"xl{pr}", name=f"xl{pr}")
                nc.scalar.dma_start(out=xl, in_=xl_d[pr * 256:(pr + 1) * 256, :]
                                    .rearrange("(b p) n -> p b n", p=128))
                xl_sb.append(xl)

            dma_w("qA", nc.gpsimd); dma_w("qB", nc.gpsimd)
            dma_x(0); dma_x(1)
            dma_w("kA", nc.gpsimd); dma_w("kB", nc.gpsimd)
            dma_x(2); dma_x(3)
            dma_w("vA", nc.gpsimd); dma_w("vB", nc.gpsimd)
            wo_sb = consts.tile([128, 4, C], F16)
            nc.gpsimd.dma_start(out=wo_sb, in_=wo_d.rearrange("p (a n) -> p a n", a=4))
            bq_sb = consts.tile([128, 4], F32)
            nc.sync.dma_start(out=bq_sb, in_=bq_d[:, :])
            ident = consts.tile([128, 128], F16)
            masks.make_identity(nc, ident[:, :])

            qT = [qkv.tile([128, 2, T], FP8, tag=f"qT{i}", name=f"qT{i}") for i in range(2)]
            kT = [qkv.tile([128, 2, T], FP8, tag=f"kT{i}", name=f"kT{i}") for i in range(2)]
            vh = [qkv.tile([128, 2, HPC, D + 1], F16, tag=f"vh{i}", name=f"vh{i}")
                  for i in range(NKT // 2)]
            for v in vh:
                nc.vector.memset(v[:, :, :, D:D + 1], 1.0)
            a_sb = [asb.tile([128, HPC, D], F16, tag=f"a{i}", name=f"a{i}")
                    for i in range(4)]
            at_sb = [asb.tile([128, 4, 256], F16, tag=f"at{i}", name=f"at{i}")
                     for i in range(2)]
            ost_sb = [asb.tile([128, 8, 256], F16, tag=f"ost{i}", name=f"ost{i}")
                      for i in range(2)]

            # ---------------- projections ----------------
            def proj(dst_kind, mt):
                # pr-outer so matmuls start as soon as x chunk pr lands
                wa, wb = w_sb[dst_kind + "A"], w_sb[dst_kind + "B"]
                engs = QUANT_Q_ENGS if dst_kind == "q" else QUANT_K_ENGS
                for t in range(4):
                    pmm = ps.tile([128, 512], F32, tag="sc", bufs=4, name="pmm")
                    n = 0
                    for (xx, ww) in ((xh_sb, wa), (xh_sb, wb), (xl_sb, wa)):
                        for pr in range(4):
                            nc.tensor.matmul(
                                pmm,
                                ww[:, pr, :, mt * 128:(mt + 1) * 128],
                                xx[pr][:, :, t * 512:(t + 1) * 512],
                                start=(n == 0), stop=(n == 11),
                                perf_mode=DR,
                            )
                            n += 1
                    sl = slice(t * 512, (t + 1) * 512)
                    dst = (qT if dst_kind == "q" else kT)[mt // 2][:, mt % 2, sl]
                    if engs[t] == "s":
                        if dst_kind == "q":
                            nc.scalar.activation(dst, pmm, IDENT,
                                                 bias=bq_sb[:, mt:mt + 1], scale=1.0 / WSC)
                        else:
                            nc.scalar.mul(dst, pmm, 1.0 / WSC)
                    elif dst_kind == "q":
                        nc.vector.tensor_scalar(dst, pmm, 1.0 / WSC,
                                                bq_sb[:, mt:mt + 1], MULT, ADD)
                    else:
                        nc.vector.tensor_scalar(dst, pmm, 1.0 / WSC, None, MULT)

            def v_proj(tt):
                pmm = ps.tile([128, 512], F32, tag="sc", bufs=4, name="pmv")
                n = 0
                for (xx, wk) in ((xh_sb, "vA"), (xh_sb, "vB"), (xl_sb, "vA")):
                    for pr in range(4):
                        nc.tensor.matmul(
                            pmm,
                            xx[pr][:, :, tt * 128:(tt + 1) * 128],
                            w_sb[wk][:, pr, :, :],
                            start=(n == 0), stop=(n == 11),
                            perf_mode=DR,
                        )
                        n += 1
                if tt % 2 == 0:
                    nc.scalar.mul(vh[tt // 2][:, tt % 2, :, 0:D],
                                  pmm.rearrange("p (a b) -> p a b", a=HPC), 1.0 / WSC)
                else:
                    nc.vector.tensor_scalar(
                        vh[tt // 2][:, tt % 2, :, 0:D],
                        pmm.rearrange("p (a b) -> p a b", a=HPC), 1.0 / WSC, None, MULT)

            # ---------------- attention ----------------
            exp_ctr = [0]

            def scores_exp(h, qc):
                """8 score tiles [128,512] (one kt-pair each) + exp -> 8 P
                tiles [128, 2, 256] per (head, qc256)."""
                ti, rb = (0, 32 * h) if h < 4 else (1, 32 * (h - 4))
                qsl = qT[ti][rb:rb + 32, :, qc * 256:(qc + 1) * 256]
                ptiles = []
                for kp in range(8):
                    sc = ps.tile([128, 512], F32, tag="sc", bufs=4, name="sc")
                    for k2 in range(2):
                        kt = kp * 2 + k2
                        nc.tensor.matmul(
                            sc[:, k2 * 256:(k2 + 1) * 256],
                            kT[ti][rb:rb + 32, :, kt * 128:(kt + 1) * 128],
                            qsl, start=True, stop=True, perf_mode=DR,
                            tile_position=(rb, 0),
                        )
                    pt = ppool.tile([128, 2, 256], F16, tag="P", bufs=80, name="pt")
                    ptiles.append(pt)
                    eng = EXP_SCHED[exp_ctr[0] % len(EXP_SCHED)]
                    exp_ctr[0] += 1
                    pf = pt.rearrange("p a b -> p (a b)")
                    if eng == "A":
                        nc.scalar.activation(pf, sc, EXPF, scale=0.125)
                    else:
                        u = pf.bitcast(I16)
                        if eng == "S":
                            nc.gpsimd.tensor_scalar(u, sc, EXP1_SCALE, PURE_BIAS, MULT, ADD)
                        elif eng == "T":
                            nc.vector.tensor_scalar(u, sc, EXP1_SCALE, PURE_BIAS, MULT, ADD)
                        else:
                            e = nc.gpsimd if eng == "L" else nc.vector
                            e.tensor_scalar(u, sc, EXP1_SCALE, EXP1_BIAS, MULT, ADD)
                            nc.vector._custom_dve(EXP_CORR, out=pf, in0=u, in1=pf,
                                                  s0=K_C, s1=A_C, imm2=B_C)
                return ptiles

            def av_chunk(pts, h, hq, qc, tqc, hi):
                av = ps.tile([128, D + 1], F32, tag="av", bufs=2, name="av")
                for kt in range(NKT):
                    nc.tensor.matmul(
                        av,
                        pts[kt // 2][:, kt % 2, tqc * 128:(tqc + 1) * 128],
                        vh[kt // 2][:, kt % 2, h, :],
                        start=(kt == 0), stop=(kt == NKT - 1),
                    )
                # normalize: approx-reciprocal of sums then per-partition mult
                rec = small.tile([128, 1], F32, tag="rec", bufs=6, name="rec")
                nc.vector.reciprocal_approx_fast(out=rec, in_=av[:, D:D + 1])
                nc.vector.tensor_scalar(
                    a_sb[(qc % 2) * 2 + tqc][:, h, :],
                    av[:, 0:D], rec, None, MULT)

            def transpose_at(qc):
                at = ps.tile([128, 4, 256], F16, tag="opo", bufs=2, name="at")
                for hp in range(4):
                    for hi in range(2):
                        h = hp * 2 + hi
                        for tqc in range(2):
                            nc.tensor.transpose(
                                at[hi * 64:hi * 64 + 64, hp, tqc * 128:(tqc + 1) * 128],
                                a_sb[(qc % 2) * 2 + tqc][:, h, :],
                                ident[:, :])
                if AT_ENGS[qc % 2] == "s":
                    nc.scalar.copy(at_sb[qc % 2], at)
                else:
                    nc.vector.tensor_copy(at_sb[qc % 2], at)

            def po_chunk(qc, mt):
                po = ps.tile([128, 256], F32, tag="opo", bufs=2, name="po")
                for hp in range(4):
                    nc.tensor.matmul(
                        po,
                        wo_sb[:, hp, mt * 128:(mt + 1) * 128],
                        at_sb[qc % 2][:, hp, :],
                        start=(hp == 0), stop=(hp == 3),
                    )
                e = PO_ENGS[(qc * 8 + mt) % len(PO_ENGS)]
                if e == "s":
                    nc.scalar.copy(ost_sb[qc % 2][:, mt, :], po)
                else:
                    ENG[e].tensor_copy(ost_sb[qc % 2][:, mt, :], po)
                if mt == 7:
                    nc.sync.dma_start(
                        out=oT_d[:, qc * 256:(qc + 1) * 256]
                        .rearrange("(a p) n -> p a n", p=128),
                        in_=ost_sb[qc % 2])

            # ---------------- emission schedule ----------------
            proj("q", 0); proj("q", 1)
            proj("k", 0); proj("k", 1)
            u0 = [scores_exp(hi, 0) for hi in range(4)]          # qc0, hq0
            proj("q", 2); proj("q", 3)
            proj("k", 2); proj("k", 3)
            u1 = [scores_exp(4 + hi, 0) for hi in range(4)]      # qc0, hq1
            for tt in range(NKT):
                v_proj(tt)

            # staggered pipeline, finely woven: consumer chunks (AV/norm,
            # transposes, outproj) of earlier units are interleaved between
            # the per-head score bursts of the current unit, so the in-order
            # PE always has work while the exp engines drain score PSUM.
            def consumers_for(done, out_qc):
                quad_p, hq, qc = done
                thunks = []
                for tqc in range(2):
                    for hi in range(4):
                        thunks.append(lambda p=quad_p[hi], h=hq * 4 + hi, hq_=hq,
                                      qc_=qc, t_=tqc, hi_=hi:
                                      av_chunk(p, h, hq_, qc_, t_, hi_))
                if hq == 1:
                    thunks.append(lambda qc_=qc: transpose_at(qc_))
                if out_qc is not None:
                    for mt in range(8):
                        thunks.append(lambda q_=out_qc, m_=mt: po_chunk(q_, m_))
                return thunks

            units = [(qc, hq) for qc in range(NQC) for hq in range(2)]
            pend = [(u0, 0, 0), (u1, 1, 0)]
            pend_out = []
            for (qc, hq) in units[2:]:
                done = pend.pop(0)
                out_qc = pend_out.pop(0) if pend_out else None
                cons = consumers_for(done, out_qc)
                if done[1] == 1:
                    pend_out.append(done[2])
                quad_p = []
                n = len(cons)
                for hi in range(4):
                    take = (n * (hi + 1)) // 4 - (n * hi) // 4
                    for _ in range(take):
                        cons.pop(0)()
                    quad_p.append(scores_exp(hq * 4 + hi, qc))
                pend.append((quad_p, hq, qc))
            for done in pend:
                out_qc = pend_out.pop(0) if pend_out else None
                for th in consumers_for(done, out_qc):
                    th()
                if done[1] == 1:
                    pend_out.append(done[2])
            for qc in pend_out:
                for mt in range(8):
                    po_chunk(qc, mt)
    nc.finalize()
    return nc


_NC = None


def _get_nc():
    global _NC
    if _NC is None:
        _NC = build_nc()
    return _NC


def _dsplit_perm():
    perm = np.empty(CS, np.int64)
    i = 0
    for mt in range(4):
        hbase, half = (0 if mt < 2 else 4), mt % 2
        for h in range(4):
            for dd in range(32):
                perm[i] = (hbase + h) * D + half * 32 + dd
                i += 1
    return perm


_PERM = _dsplit_perm()


def _w_pair_layout(w):
    return np.ascontiguousarray(
        w.reshape(4, 2, 128, CS).transpose(2, 0, 1, 3).reshape(128, 4 * 2 * CS))


def _quant_w(wT):
    wa = (WSC * wT).astype(E4M3)
    wb = (WSC * wT - wa.astype(np.float32)).astype(E4M3)
    return _w_pair_layout(wa), _w_pair_layout(wb)


def _shard_inputs(x, Wq, bq, Wk, bk, Wv, bv, Wo, bo):
    x = np.asarray(x, np.float32)
    wqT = np.ascontiguousarray(np.asarray(Wq, np.float32).T)
    wkT = np.ascontiguousarray(np.asarray(Wk, np.float32).T)
    wvT = np.ascontiguousarray(np.asarray(Wv, np.float32).T)
    woT = np.ascontiguousarray(np.asarray(Wo, np.float32).T)
    bqf = np.asarray(bq, np.float32)

    xh_b, xl_b = [], []
    for b in range(B):
        xT = np.ascontiguousarray(x[b].T)
        xh = xT.astype(E4M3)
        xl = (xT - xh.astype(np.float32)).astype(E4M3)
        xh_b.append(xh); xl_b.append(xl)

    in_maps = []
    for c in range(8):
        b, g = c % B, c // B
        sl = slice(g * CS, (g + 1) * CS)
        qa, qb = _quant_w(wqT[:, sl][:, _PERM])
        ka, kb = _quant_w(wkT[:, sl][:, _PERM])
        va, vb_ = _quant_w(wvT[:, sl])
        wos = np.ascontiguousarray(
            woT[sl, :].reshape(4, 128, C).transpose(1, 0, 2)
            .reshape(128, 4 * C)).astype(np.float16)
        bqp = np.ascontiguousarray(
            bqf[sl][_PERM].reshape(4, 128).T).astype(np.float32)
        in_maps.append({
            "xh": xh_b[b], "xl": xl_b[b],
            "wqA": qa, "wqB": qb, "wkA": ka, "wkB": kb, "wvA": va, "wvB": vb_,
            "woT": wos, "bq": bqp,
        })
    return in_maps


def run_sharded(inputs, **kwargs):
    nc = _get_nc()
    in_maps = _shard_inputs(**inputs)
    return run_bass_kernel_spmd(nc, in_maps, core_ids=list(range(8)), **kwargs)


def assemble(results, Wv_bias, Wo, bo):
    bo_eff = (np.asarray(bo, np.float32)
              + np.asarray(Wo, np.float32) @ np.asarray(Wv_bias, np.float32))
    out = np.empty((B, T, C), np.float32)
    for b in range(B):
        acc = results[b]["oT"].astype(np.float32) + results[b + B]["oT"].astype(np.float32)
        out[b] = acc.T + bo_eff[None, :]
    return out


def kernel(**inputs):
    res = run_sharded(inputs)
    return assemble(res.results, inputs["bv"], inputs["Wo"], inputs["bo"])
